# revision 5
# baseline (speedup 1.0000x reference)
"""GRUCell fused kernel for Trainium2, data-parallel over 8 NeuronCores.

Strategy:
  - Shard batch (16384) across 8 cores -> 2048 rows/core; replicate weights.
  - Host-side: feed activations feature-major (x.T, h.T per shard) and
    weights packed per output j-tile in exact consumption order, so the
    device never transposes anything and the PE pipeline starts after
    ~1.5MB of DMA instead of the full 6MB weight set.
  - Device: out.T tiles [128 h-units, 512 batch] computed as
    W.T-slices (stationary) x act.T (moving) matmuls in float32r
    (1 cycle/row at N=512; true fp32 is 4x slower), fp32 PSUM accumulate.
    Gate order ig -> hg -> r -> z so early gates only need small weights.
    Epilogue on ACT (sigmoid/tanh with fused bias) + DVE
    (scalar_tensor_tensor to fold remaining biases).
  - h' = n + z*(h - n) where n = tanh(i_g + r*h_g).
"""

import os
import numpy as np
from contextlib import ExitStack

import concourse.bass as bass
import concourse.tile as tile
from concourse import bacc, mybir
from concourse.bass_utils import run_bass_kernel_spmd

B, I, H = 16384, 512, 512
NCORES = 8
BL = B // NCORES          # 2048 rows per core
NB = 512                  # batch tile (matmul moving free dim)
NBT = BL // NB            # 4 batch tiles per core
P = 128                   # partitions
KX = I // P               # 4 k-tiles over input features
KH = H // P               # 4 k-tiles over hidden features
JT = H // P               # 4 output j-tiles per gate

FP32 = mybir.dt.float32
FP32R = mybir.dt.float32r

_cache = {}


def build_gru_bass():
    """Build (once) the SPMD Bass program for one core's shard."""
    if "nc" in _cache:
        return _cache["nc"]

    nc = bacc.Bacc(
        "TRN2",
        target_bir_lowering=False,
        debug=False,
        enable_asserts=False,
        num_devices=NCORES,
    )

    xT = nc.dram_tensor("xT", [I, BL], FP32R, kind="ExternalInput").ap()
    hT = nc.dram_tensor("hT", [H, BL], FP32R, kind="ExternalInput").ap()
    # packed weights per j-tile: [JT, 128, 3072]; free-dim column groups:
    #   [0:512)      w_i blocks kt=0..3   (W_i.T  [kt*128:+128, jt*128:+128])
    #   [512:1024)   w_h blocks kt=0..3
    #   [1024:2048)  w_r blocks kt=0..7   (W_gate.T cols jt*128:+128)
    #   [2048:3072)  w_z blocks kt=0..7   (W_gate.T cols 512+jt*128:+128)
    wpk = nc.dram_tensor("wpk", [JT, P, 3072], FP32R, kind="ExternalInput").ap()
    # bias columns: 0..3 b_r per j-tile, 4..7 b_z, 8..11 b_i, 12..15 b_h
    bias = nc.dram_tensor("bias", [P, 16], FP32, kind="ExternalInput").ap()
    outT = nc.dram_tensor("outT", [H, BL], FP32, kind="ExternalOutput").ap()

    ADD = mybir.AluOpType.add
    MULT = mybir.AluOpType.mult
    SIG = mybir.ActivationFunctionType.Sigmoid
    TANH = mybir.ActivationFunctionType.Tanh

    with tile.TileContext(nc) as tc, ExitStack() as ctx:
        wpool = ctx.enter_context(tc.tile_pool(name="weights", bufs=1))
        apool = ctx.enter_context(tc.tile_pool(name="acts", bufs=2))
        ppool = ctx.enter_context(tc.tile_pool(name="psum", bufs=2, space="PSUM"))
        epool = ctx.enter_context(tc.tile_pool(name="epi", bufs=3))

        bias_s = wpool.tile([P, 16], FP32, tag="bias", name="bias_s")
        nc.sync.dma_start(out=bias_s[:], in_=bias[:, :])

        # bt0 activations first: the first matmuls only need xt + w_i of jt0.
        xt_all = [[None] * KX for _ in range(NBT)]
        ht_all = [[None] * KH for _ in range(NBT)]

        def load_acts(bt):
            bsl = bass.ts(bt, NB)
            for kt in range(KX):
                xtile = apool.tile([P, NB], FP32R, tag=f"xt{kt}",
                                   name=f"xt{kt}_{bt}")
                nc.sync.dma_start(out=xtile[:], in_=xT[kt * P:(kt + 1) * P, bsl])
                xt_all[bt][kt] = xtile
            for kt in range(KH):
                htile = apool.tile([P, NB], FP32R, tag=f"ht{kt}",
                                   name=f"ht{kt}_{bt}")
                nc.sync.dma_start(out=htile[:], in_=hT[kt * P:(kt + 1) * P, bsl])
                ht_all[bt][kt] = htile

        # Interleave the first batch-tile's activation loads with jt0's
        # weight groups in exact first-use order, so the first matmul's
        # inputs complete after ~0.8MB of DMA instead of ~2.5MB.
        w_i, w_h, w_r, w_z = [None] * JT, [None] * JT, [None] * JT, [None] * JT

        def load_wgroup(jt, which):
            col0 = {"i": 0, "h": 512, "r": 1024, "z": 2048}[which]
            width = 512 if which in ("i", "h") else 1024
            wt = wpool.tile([P, width], FP32R, tag=f"w{which}{jt}",
                            name=f"w{which}{jt}")
            nc.sync.dma_start(out=wt[:], in_=wpk[jt, :, col0:col0 + width])
            {"i": w_i, "h": w_h, "r": w_r, "z": w_z}[which][jt] = wt

        bsl0 = bass.ts(0, NB)
        # jt0's w_i arrives in per-kt 128KB pieces written into one tile's
        # column slices, each paired with the xt k-tile it multiplies, so
        # matmul #1 only waits on ~384KB of DMA.
        wi0 = wpool.tile([P, 512], FP32R, tag="wi0", name="wi0")
        for kt in range(KX):
            xtile = apool.tile([P, NB], FP32R, tag=f"xt{kt}", name=f"xt{kt}_0")
            nc.sync.dma_start(out=xtile[:], in_=xT[kt * P:(kt + 1) * P, bsl0])
            xt_all[0][kt] = xtile
            nc.sync.dma_start(out=wi0[:, kt * P:(kt + 1) * P],
                              in_=wpk[0, :, kt * P:(kt + 1) * P])
        w_i[0] = wi0
        for kt in range(KH):
            htile = apool.tile([P, NB], FP32R, tag=f"ht{kt}", name=f"ht{kt}_0")
            nc.sync.dma_start(out=htile[:], in_=hT[kt * P:(kt + 1) * P, bsl0])
            ht_all[0][kt] = htile
        load_wgroup(0, "h")
        load_wgroup(0, "r")
        load_wgroup(0, "z")
        for jt in range(1, JT):
            for which in ("i", "h", "r", "z"):
                load_wgroup(jt, which)

        # ---- main loop over batch tiles ----
        for bt in range(NBT):
            bsl = bass.ts(bt, NB)
            if bt > 0:
                load_acts(bt)
            xt = xt_all[bt]
            ht = ht_all[bt]

            for jt in range(JT):
                j0 = jt * P
                # i_gate pre-activation: K = I
                ig_ps = ppool.tile([P, NB], FP32, tag="ig_ps", name=f"ig_ps_{bt}_{jt}")
                for kt in range(KX):
                    nc.tensor.matmul(
                        out=ig_ps[:], lhsT=w_i[jt][:, kt * P:(kt + 1) * P],
                        rhs=xt[kt][:], start=(kt == 0), stop=(kt == KX - 1))
                # h_gate pre-activation: K = H
                hg_ps = ppool.tile([P, NB], FP32, tag="hg_ps", name=f"hg_ps_{bt}_{jt}")
                for kt in range(KH):
                    nc.tensor.matmul(
                        out=hg_ps[:], lhsT=w_h[jt][:, kt * P:(kt + 1) * P],
                        rhs=ht[kt][:], start=(kt == 0), stop=(kt == KH - 1))
                # r gate pre-activation: K = I + H
                r_ps = ppool.tile([P, NB], FP32, tag="r_ps", name=f"r_ps_{bt}_{jt}")
                for kt in range(KX):
                    nc.tensor.matmul(
                        out=r_ps[:], lhsT=w_r[jt][:, kt * P:(kt + 1) * P],
                        rhs=xt[kt][:], start=(kt == 0), stop=False)
                for kt in range(KH):
                    nc.tensor.matmul(
                        out=r_ps[:], lhsT=w_r[jt][:, (KX + kt) * P:(KX + kt + 1) * P],
                        rhs=ht[kt][:], start=False, stop=(kt == KH - 1))
                # z gate pre-activation: K = I + H
                z_ps = ppool.tile([P, NB], FP32, tag="z_ps", name=f"z_ps_{bt}_{jt}")
                for kt in range(KX):
                    nc.tensor.matmul(
                        out=z_ps[:], lhsT=w_z[jt][:, kt * P:(kt + 1) * P],
                        rhs=xt[kt][:], start=(kt == 0), stop=False)
                for kt in range(KH):
                    nc.tensor.matmul(
                        out=z_ps[:], lhsT=w_z[jt][:, (KX + kt) * P:(KX + kt + 1) * P],
                        rhs=ht[kt][:], start=False, stop=(kt == KH - 1))

                # ---- epilogue ----
                r_s = epool.tile([P, NB], FP32, tag="r_s", name=f"r_s_{bt}_{jt}")
                nc.scalar.activation(out=r_s[:], in_=r_ps[:], func=SIG,
                                     bias=bias_s[:, jt:jt + 1])
                z_s = epool.tile([P, NB], FP32, tag="z_s", name=f"z_s_{bt}_{jt}")
                nc.scalar.activation(out=z_s[:], in_=z_ps[:], func=SIG,
                                     bias=bias_s[:, 4 + jt:5 + jt])
                # m = (h_gate + b_h) * r
                m = epool.tile([P, NB], FP32, tag="m", name=f"m_{bt}_{jt}")
                nc.vector.scalar_tensor_tensor(
                    out=m[:], in0=hg_ps[:], scalar=bias_s[:, 12 + jt:13 + jt],
                    in1=r_s[:], op0=ADD, op1=MULT)
                # s = (i_gate + b_i) + m
                s = epool.tile([P, NB], FP32, tag="s", name=f"s_{bt}_{jt}")
                nc.vector.scalar_tensor_tensor(
                    out=s[:], in0=ig_ps[:], scalar=bias_s[:, 8 + jt:9 + jt],
                    in1=m[:], op0=ADD, op1=ADD)
                n = epool.tile([P, NB], FP32, tag="n", name=f"n_{bt}_{jt}")
                nc.scalar.activation(out=n[:], in_=s[:], func=TANH)
                # out = n + z * (h - n)
                d = epool.tile([P, NB], FP32, tag="d", name=f"d_{bt}_{jt}")
                nc.vector.tensor_sub(d[:], ht[jt][:].bitcast(FP32), n[:])
                e = epool.tile([P, NB], FP32, tag="e", name=f"e_{bt}_{jt}")
                nc.vector.tensor_mul(e[:], z_s[:], d[:])
                o = epool.tile([P, NB], FP32, tag="o", name=f"o_{bt}_{jt}")
                nc.vector.tensor_add(o[:], n[:], e[:])
                nc.sync.dma_start(out=outT[j0:j0 + P, bsl], in_=o[:])

    nc.compile()
    _cache["nc"] = nc
    return nc


def kernel(input, hidden, W_gate, b_gate, W_i, b_i, W_h, b_h):
    input = np.asarray(input, dtype=np.float32)
    hidden = np.asarray(hidden, dtype=np.float32)
    W_gate = np.asarray(W_gate, dtype=np.float32)
    b_gate = np.asarray(b_gate, dtype=np.float32)
    W_i = np.asarray(W_i, dtype=np.float32)
    b_i = np.asarray(b_i, dtype=np.float32)
    W_h = np.asarray(W_h, dtype=np.float32)
    b_h = np.asarray(b_h, dtype=np.float32)

    nc = build_gru_bass()

    wgT = W_gate.T            # [I+H, 2H]
    wiT = W_i.T               # [I, H]
    whT = W_h.T               # [H, H]
    wpk = np.empty((JT, P, 3072), dtype=np.float32)
    for jt in range(JT):
        jsl = slice(jt * P, (jt + 1) * P)
        for kt in range(KX):
            wpk[jt, :, kt * P:(kt + 1) * P] = wiT[kt * P:(kt + 1) * P, jsl]
        for kt in range(KH):
            wpk[jt, :, 512 + kt * P:512 + (kt + 1) * P] = \
                whT[kt * P:(kt + 1) * P, jsl]
        for kt in range(KX + KH):
            wpk[jt, :, 1024 + kt * P:1024 + (kt + 1) * P] = \
                wgT[kt * P:(kt + 1) * P, jsl]
            wpk[jt, :, 2048 + kt * P:2048 + (kt + 1) * P] = \
                wgT[kt * P:(kt + 1) * P, H + jt * P:H + (jt + 1) * P]
    # bias pack: [128, 16]; column layout r(4) z(4) i(4) h(4), col jt holds
    # bias[jt*128:(jt+1)*128]
    bias = np.concatenate([
        b_gate[:H].reshape(JT, P).T,
        b_gate[H:].reshape(JT, P).T,
        b_i.reshape(JT, P).T,
        b_h.reshape(JT, P).T,
    ], axis=1).astype(np.float32)
    bias = np.ascontiguousarray(bias)

    in_maps = []
    for c in range(NCORES):
        sl = slice(c * BL, (c + 1) * BL)
        in_maps.append({
            "xT": np.ascontiguousarray(input[sl].T),
            "hT": np.ascontiguousarray(hidden[sl].T),
            "wpk": wpk,
            "bias": bias,
        })

    res = run_bass_kernel_spmd(
        nc, in_maps, list(range(NCORES)),
        trace=bool(int(os.environ.get("GRU_TRACE", "0"))),
    )
    out = np.empty((B, H), dtype=np.float32)
    for c in range(NCORES):
        out[c * BL:(c + 1) * BL, :] = res.results[c]["outT"].T
    if res.exec_time_ns is not None:
        kernel.last_exec_time_ns = res.exec_time_ns
        kernel.last_results = res
    return out


kernel.last_exec_time_ns = None
kernel.last_results = None
